# revision 8
# baseline (speedup 1.0000x reference)
"""Bass/Trainium2 kernel for nn_BipartPool: bipartite attention pooling.

Math (B=64 graphs, N=128 nodes/graph, R=32 aggregator queries/graph,
H=8 heads, HD=64, E=512):
  q = (aggrs @ Wq.T + bq)            -- identical for every graph
  k = x @ Wk.T, v = x @ Wv.T          (per node)
  per graph g, head h: attn = softmax(q_h k_{g,h}^T / sqrt(HD))
  out_g = concat_h(attn @ v_{g,h}) @ Wo.T + bo

Sharding: data-parallel over graphs, 8 graphs per core across 8 cores.
Weights replicated. No collectives.

Simplifications used (all mathematically exact):
  - k-bias bk drops out (softmax shift invariance along the node axis).
  - v-bias bv folds into the output bias: bo_eff = Wo @ bv + bo.
  - 1/sqrt(HD) folds into Wq and bq host-side.
  - softmax without max-subtraction (scores are ~N(0,1); exp is safe in fp32).

Device layout per core (G=8 graphs, S=G*128=1024 nodes):
  xT   [E, S]  (host-transposed)   KT = Wk @ xT   [E_f, S]
  V    [S, E]  (natural)           QT  [E_f, 32]
  Qblk [128, 64] per f-chunk: block-diag pair of per-head qT -> one
      matmul per (head-pair, s-half) produces scores for 2 heads.
  softmax rows = (head, query) on partitions, nodes on free axis.
  PE-transpose of normalized attn -> attnT [node, (4 heads x 32 q)].
  outT_h = v_{g,h}.T-free matmul(lhsT=v, rhs=attnT slice) -> yT chunks.
  out = yT.T @ WoT (+ ones x bo_eff outer product) -> [256, 512] per core.
"""

import numpy as np

import concourse.bacc as bacc
import concourse.mybir as mybir
from concourse import tile
from concourse.bass_utils import run_bass_kernel_spmd

F32 = mybir.dt.float32
F32R = mybir.dt.float32r
AF = mybir.ActivationFunctionType

B, N, RATIO, H, HD = 64, 128, 32, 8, 64
E = H * HD                 # 512
NCORES = 8
G = B // NCORES            # 8 graphs per core
S = G * N                  # 1024 nodes per core
EC = E // 128              # 4 e-chunks (contraction)
FC = E // 128              # 4 f-chunks (output features)

_CACHE = {}
LAST_RESULT = None         # test harness reads exec_time_ns from here


def _r(ap):
    return ap.bitcast(F32R)


def _emit(nc, tc, d):
    """Emit the per-core program. d: dict of dram APs."""
    with (
        nc.allow_low_precision(reason="float32r rounding is intended"),
        tc.tile_pool(name="sb", bufs=1) as sb,
        tc.tile_pool(name="ps", bufs=3, space="PSUM") as ps,
        tc.tile_pool(name="ps2", bufs=4, space="PSUM") as ps2,
    ):
        # ---- persistent SBUF tensors -------------------------------------
        x_sb = sb.tile([128, EC, S], F32R)          # xT  [e-part, ec, s]
        wk_sb = sb.tile([128, EC, E], F32R)         # WkT [e-part, ec, f]
        wv_sb = sb.tile([128, EC, E], F32R)
        wq_sb = sb.tile([128, EC, E], F32R)
        wo_sb = sb.tile([128, FC, E], F32R)         # WoT [f-part, fc, e]
        ag_sb = sb.tile([128, EC, RATIO], F32R)     # aggrsT [e-part, ec, q]
        bq_sb = sb.tile([128, FC, 1], F32)
        bo_sb = sb.tile([1, E], F32R)
        id_sb = sb.tile([128, 128], F32R)           # identity for PE transpose
        kt_sb = sb.tile([128, FC, S], F32R)         # KT [f-part, fc, s]
        v_sb = sb.tile([128, G, E], F32R)           # V  [node, g, f]
        qb_sb = sb.tile([128, FC, 64], F32R)        # block-diag head-pair qT
        ex_sb = sb.tile([128, 4, 512], F32R)        # exp(scores) (hgrp, sh)
        at_sb = sb.tile([128, 2, G, 128], F32R)     # attnT (hgrp, g)
        y_sb = sb.tile([128, FC, 2, 128], F32R)     # yT (head-pair, gg)
        o_sb = sb.tile([128, 2, E], F32)           # output rows
        den_sb = sb.tile([128, 4, 4], F32)         # softmax denominators
        rec_sb = sb.tile([128, 4, 4], F32)
        ones_sb = sb.tile([1, 128], F32R)

        # ---- DMA in ------------------------------------------------------
        nc.sync.dma_start(out=x_sb[:], in_=d["xT"].rearrange("(c p) s -> p c s", p=128))
        nc.sync.dma_start(out=wk_sb[:], in_=d["wkT"].rearrange("(c p) f -> p c f", p=128))
        nc.sync.dma_start(out=wv_sb[:], in_=d["wvT"].rearrange("(c p) f -> p c f", p=128))
        nc.sync.dma_start(out=wq_sb[:], in_=d["wqT"].rearrange("(c p) f -> p c f", p=128))
        nc.sync.dma_start(out=wo_sb[:], in_=d["woT"].rearrange("(c p) e -> p c e", p=128))
        nc.sync.dma_start(out=ag_sb[:], in_=d["agT"].rearrange("(c p) q -> p c q", p=128))
        nc.sync.dma_start(out=bq_sb[:], in_=d["bq"].rearrange("(c p) o -> p c o", p=128))
        nc.sync.dma_start(out=bo_sb[:], in_=d["bo"][:])
        nc.sync.dma_start(out=id_sb[:], in_=d["ident"][:])
        nc.sync.dma_start(out=ones_sb[:], in_=d["ones"][:])

        # ---- Q projection: QT[f,q] = sum_e WqT[e,f] agT[e,q] -------------
        for fc in range(FC):
            qp = ps.tile([128, RATIO], F32, tag="mm512", name=f"qp{fc}")
            for ec in range(EC):
                nc.tensor.matmul(
                    qp[:],
                    (wq_sb[:, ec, fc * 128:(fc + 1) * 128]),
                    (ag_sb[:, ec, :]),
                    start=(ec == 0), stop=(ec == EC - 1),
                )
            # block-diag pack (+ q bias): rows 0-63 head 2fc -> cols 0-31,
            # rows 64-127 head 2fc+1 -> cols 32-63
            nc.scalar.activation(qb_sb[0:64, fc, 0:RATIO], qp[0:64, :], AF.Identity,
                                 bias=bq_sb[0:64, fc, :])
            nc.scalar.activation(qb_sb[64:128, fc, RATIO:64], qp[64:128, :], AF.Identity,
                                 bias=bq_sb[64:128, fc, :])
            nc.scalar.activation(qb_sb[0:64, fc, RATIO:64], qp[0:64, :], AF.Identity,
                                 scale=0.0)
            nc.scalar.activation(qb_sb[64:128, fc, 0:RATIO], qp[64:128, :], AF.Identity,
                                 scale=0.0)

        # ---- K projection: KT[f,s] (f-chunk fc, s-half sh) ---------------
        for fc in range(FC):
            for sh in range(2):
                kp = ps.tile([128, 512], F32, tag="mm512", name=f"kp{fc}{sh}")
                for ec in range(EC):
                    nc.tensor.matmul(
                        kp[:],
                        (wk_sb[:, ec, fc * 128:(fc + 1) * 128]),
                        (x_sb[:, ec, sh * 512:(sh + 1) * 512]),
                        start=(ec == 0), stop=(ec == EC - 1),
                    )
                nc.vector.tensor_copy(kt_sb[:, fc, sh * 512:(sh + 1) * 512], kp[:])

        # ---- V projection: V[s,f] per graph ------------------------------
        for g in range(G):
            vp = ps.tile([128, 512], F32, tag="mm512", name=f"vp{g}")
            for ec in range(EC):
                nc.tensor.matmul(
                    vp[:],
                    (x_sb[:, ec, g * 128:(g + 1) * 128]),
                    (wv_sb[:, ec, :]),
                    start=(ec == 0), stop=(ec == EC - 1),
                )
            nc.vector.tensor_copy(v_sb[:, g, :], vp[:])

        # ---- scores + softmax + transpose --------------------------------
        # scores tile t=(hgrp, sh): [128 rows = 4 heads x 32 q, 512 = 4 graphs x 128 nodes]
        for hgrp in range(2):
            for sh in range(2):
                t = hgrp * 2 + sh
                for half in range(2):
                    fc = hgrp * 2 + half
                    sp = ps.tile([64, 512], F32, tag="mm512", name=f"sp{t}{half}")
                    nc.tensor.matmul(
                        sp[:],
                        (qb_sb[:, fc, :]),
                        (kt_sb[:, fc, sh * 512:(sh + 1) * 512]),
                        start=True, stop=True,
                    )
                    nc.scalar.activation(
                        ex_sb[half * 64:(half + 1) * 64, t, :], sp[:], AF.Exp)
                nc.vector.reduce_sum(
                    den_sb[:, t, :],
                    ex_sb[:, t, :].rearrange("p (j n) -> p j n", n=128),
                    axis=mybir.AxisListType.X,
                )
                nc.vector.reciprocal(rec_sb[:, t, :], den_sb[:, t, :])
                for j in range(4):
                    g = sh * 4 + j
                    nc.vector.tensor_scalar_mul(
                        ex_sb[:, t, j * 128:(j + 1) * 128],
                        ex_sb[:, t, j * 128:(j + 1) * 128],
                        rec_sb[:, t, j:j + 1],
                    )
                    tp = ps2.tile([128, 128], F32, tag="mm128", name=f"tp{t}{j}")
                    nc.tensor.transpose(tp[:].bitcast(F32R),
                                        (ex_sb[:, t, j * 128:(j + 1) * 128]),
                                        (id_sb[:]))
                    nc.vector.tensor_copy(at_sb[:, hgrp, g, :], tp[:])

        # ---- attention output: yT[f=(2 heads x 64 d), (4 g x 32 q)] ------
        for gg in range(2):
            for hp in range(FC):          # head-pair hp: heads (2hp, 2hp+1)
                for hh in range(2):
                    h = 2 * hp + hh
                    hgrp, hl = h // 4, h % 4
                    yp = ps2.tile([64, 128], F32, tag="mm128", name=f"yp{gg}{h}")
                    for jg in range(4):
                        g = gg * 4 + jg
                        nc.tensor.matmul(
                            yp[:, jg * 32:(jg + 1) * 32],
                            (v_sb[:, g, h * 64:(h + 1) * 64]),
                            (at_sb[:, hgrp, g, hl * 32:(hl + 1) * 32]),
                            start=True, stop=True,
                        )
                    nc.vector.tensor_copy(y_sb[hh * 64:(hh + 1) * 64, hp, gg, :], yp[:])

        # ---- output projection + bias ------------------------------------
        for gg in range(2):
            op = ps.tile([128, 512], F32, tag="mm512", name=f"op{gg}")
            for hp in range(FC):
                nc.tensor.matmul(
                    op[:], (y_sb[:, hp, gg, :]), (wo_sb[:, hp, :]),
                    start=(hp == 0), stop=False,
                )
            nc.tensor.matmul(op[:], (ones_sb[:]), (bo_sb[:]),
                             start=False, stop=True)
            nc.vector.tensor_copy(o_sb[:, gg, :], op[:])
            nc.sync.dma_start(out=d["out"][gg * 128:(gg + 1) * 128, :],
                              in_=o_sb[:, gg, :])


def _build():
    nc = bacc.Bacc("TRN2", target_bir_lowering=False, debug=False)
    d = {}
    d["xT"] = nc.dram_tensor("xT", (E, S), F32R, kind="ExternalInput").ap()
    d["wkT"] = nc.dram_tensor("wkT", (E, E), F32R, kind="ExternalInput").ap()
    d["wvT"] = nc.dram_tensor("wvT", (E, E), F32R, kind="ExternalInput").ap()
    d["wqT"] = nc.dram_tensor("wqT", (E, E), F32R, kind="ExternalInput").ap()
    d["woT"] = nc.dram_tensor("woT", (E, E), F32R, kind="ExternalInput").ap()
    d["agT"] = nc.dram_tensor("agT", (E, RATIO), F32R, kind="ExternalInput").ap()
    d["bq"] = nc.dram_tensor("bq", (E, 1), F32, kind="ExternalInput").ap()
    d["bo"] = nc.dram_tensor("bo", (1, E), F32R, kind="ExternalInput").ap()
    d["ident"] = nc.dram_tensor("ident", (128, 128), F32R, kind="ExternalInput").ap()
    d["ones"] = nc.dram_tensor("ones", (1, 128), F32R, kind="ExternalInput").ap()
    d["out"] = nc.dram_tensor("out", (G * RATIO, E), F32, kind="ExternalOutput").ap()
    with tile.TileContext(nc) as tc:
        _emit(nc, tc, d)
    nc.compile()
    return nc


def kernel(x, batch, aggrs, in_proj_w, in_proj_b, out_proj_w, out_proj_b):
    global LAST_RESULT
    x = np.ascontiguousarray(np.asarray(x, dtype=np.float32))
    aggrs = np.asarray(aggrs, dtype=np.float32)
    in_proj_w = np.asarray(in_proj_w, dtype=np.float32)
    in_proj_b = np.asarray(in_proj_b, dtype=np.float32)
    out_proj_w = np.asarray(out_proj_w, dtype=np.float32)
    out_proj_b = np.asarray(out_proj_b, dtype=np.float32)

    scale = np.float32(1.0 / np.sqrt(HD))
    wq, wk, wv = in_proj_w[:E], in_proj_w[E:2 * E], in_proj_w[2 * E:]
    bq = in_proj_b[:E] * scale
    bv = in_proj_b[2 * E:]
    wqT = np.ascontiguousarray((wq * scale).T)
    wkT = np.ascontiguousarray(wk.T)
    wvT = np.ascontiguousarray(wv.T)
    woT = np.ascontiguousarray(out_proj_w.T)
    agT = np.ascontiguousarray(aggrs.T)
    bo_eff = (out_proj_w @ bv + out_proj_b).reshape(1, E)
    ident = np.eye(128, dtype=np.float32)

    shared = {
        "wkT": wkT, "wvT": wvT, "wqT": wqT, "woT": woT, "agT": agT,
        "bq": bq.reshape(E, 1).astype(np.float32),
        "bo": bo_eff.astype(np.float32),
        "ident": ident,
        "ones": np.ones((1, 128), dtype=np.float32),
    }
    in_maps = []
    for c in range(NCORES):
        xc = x[c * G:(c + 1) * G].reshape(S, E)
        m = dict(shared)
        m["xT"] = np.ascontiguousarray(xc.T)
        in_maps.append(m)

    if "nc" not in _CACHE:
        _CACHE["nc"] = _build()
    nc = _CACHE["nc"]

    res = run_bass_kernel_spmd(nc, in_maps, list(range(NCORES)))
    LAST_RESULT = res
    out = np.concatenate([res.results[c]["out"] for c in range(NCORES)], axis=0)
    return out.reshape(B, RATIO, E).astype(np.float32)


# revision 10
# speedup vs baseline: 1.1191x; 1.1191x over previous
"""Bass/Trainium2 kernel for nn_BipartPool: bipartite attention pooling.

Math (B=64 graphs, N=128 nodes/graph, R=32 aggregator queries/graph,
H=8 heads, HD=64, E=512):
  q = (aggrs @ Wq.T + bq)            -- identical for every graph
  k = x @ Wk.T, v = x @ Wv.T          (per node)
  per graph g, head h: attn = softmax(q_h k_{g,h}^T / sqrt(HD))
  out_g = concat_h(attn @ v_{g,h}) @ Wo.T + bo

Sharding: data-parallel over graphs, 8 graphs per core across 8 cores.
Weights replicated. No collectives.

Simplifications used (all mathematically exact):
  - k-bias bk drops out (softmax shift invariance along the node axis).
  - v-bias bv folds into the output bias: bo_eff = Wo @ bv + bo.
  - 1/sqrt(HD) folds into Wq and bq host-side.
  - softmax without max-subtraction (scores are ~N(0,1); exp is safe in fp32).

Device layout per core (G=8 graphs, S=G*128=1024 nodes):
  xT   [E, S]  (host-transposed)   KT = Wk @ xT   [E_f, S]
  V    [S, E]  (natural)           QT  [E_f, 32]
  Qblk [128, 64] per f-chunk: block-diag pair of per-head qT -> one
      matmul per (head-pair, s-half) produces scores for 2 heads.
  softmax rows = (head, query) on partitions, nodes on free axis.
  PE-transpose of normalized attn -> attnT [node, (4 heads x 32 q)].
  outT_h = v_{g,h}.T-free matmul(lhsT=v, rhs=attnT slice) -> yT chunks.
  out = yT.T @ WoT (+ ones x bo_eff outer product) -> [256, 512] per core.
"""

import numpy as np

import concourse.bacc as bacc
import concourse.mybir as mybir
from concourse import tile
from concourse.bass_utils import run_bass_kernel_spmd

F32 = mybir.dt.float32
F32R = mybir.dt.float32r
AF = mybir.ActivationFunctionType

B, N, RATIO, H, HD = 64, 128, 32, 8, 64
E = H * HD                 # 512
NCORES = 8
G = B // NCORES            # 8 graphs per core
S = G * N                  # 1024 nodes per core
EC = E // 128              # 4 e-chunks (contraction)
FC = E // 128              # 4 f-chunks (output features)

_CACHE = {}
LAST_RESULT = None         # test harness reads exec_time_ns from here


def _r(ap):
    return ap.bitcast(F32R)


def _emit(nc, tc, d):
    """Emit the per-core program. d: dict of dram APs."""
    with (
        nc.allow_low_precision(reason="float32r rounding is intended"),
        tc.tile_pool(name="sb", bufs=1) as sb,
        tc.tile_pool(name="ps", bufs=4, space="PSUM") as ps,
        tc.tile_pool(name="ps2", bufs=4, space="PSUM") as ps2,
    ):
        # ---- persistent SBUF tensors -------------------------------------
        x_sb = sb.tile([128, EC, S], F32R)          # xT  [e-part, ec, s]
        wk_sb = sb.tile([128, EC, E], F32R)         # WkT [e-part, ec, f]
        wv_sb = sb.tile([128, EC, E], F32R)
        wq_sb = sb.tile([128, EC, E], F32R)
        wo_sb = sb.tile([128, FC, E], F32R)         # WoT [f-part, fc, e]
        ag_sb = sb.tile([128, EC, RATIO], F32R)     # aggrsT [e-part, ec, q]
        bq_sb = sb.tile([128, FC, 1], F32)
        bo_sb = sb.tile([1, E], F32R)
        id_sb = sb.tile([128, 128], F32R)           # identity for PE transpose
        kt_sb = sb.tile([128, FC, S], F32R)         # KT [f-part, fc, s]
        v_sb = sb.tile([128, G, E], F32R)           # V  [node, g, f]
        qb_sb = sb.tile([128, FC, 64], F32R)        # block-diag head-pair qT
        ex_sb = sb.tile([128, 4, 512], F32R)        # exp(scores) (hgrp, sh)
        at_sb = sb.tile([128, 2, G, 128], F32R)     # attnT (hgrp, g)
        y_sb = sb.tile([128, FC, 2, 128], F32R)     # yT (head-pair, gg)
        o_sb = sb.tile([128, 2, E], F32)           # output rows
        den_sb = sb.tile([128, 4, 4], F32)         # softmax denominators
        rec_sb = sb.tile([128, 4, 4], F32)
        ones_sb = sb.tile([1, 128], F32R)

        # ---- DMA in ------------------------------------------------------
        nc.sync.dma_start(out=ag_sb[:], in_=d["agT"].rearrange("(c p) q -> p c q", p=128))
        nc.sync.dma_start(out=bq_sb[:], in_=d["bq"].rearrange("(c p) o -> p c o", p=128))
        nc.sync.dma_start(out=wq_sb[:], in_=d["wqT"].rearrange("(c p) f -> p c f", p=128))
        nc.gpsimd.dma_start(out=x_sb[:], in_=d["xT"].rearrange("(c p) s -> p c s", p=128))
        nc.scalar.dma_start(out=wk_sb[:], in_=d["wkT"].rearrange("(c p) f -> p c f", p=128))
        nc.scalar.dma_start(out=wv_sb[:], in_=d["wvT"].rearrange("(c p) f -> p c f", p=128))
        nc.gpsimd.dma_start(out=id_sb[:], in_=d["ident"][:])
        nc.sync.dma_start(out=wo_sb[:], in_=d["woT"].rearrange("(c p) e -> p c e", p=128))
        nc.sync.dma_start(out=bo_sb[:], in_=d["bo"][:])
        nc.sync.dma_start(out=ones_sb[:], in_=d["ones"][:])

        # ---- Q projection: QT[f,q] = sum_e WqT[e,f] agT[e,q] -------------
        for fc in range(FC):
            qp = ps.tile([128, RATIO], F32, tag="mm512", name=f"qp{fc}")
            for ec in range(EC):
                nc.tensor.matmul(
                    qp[:],
                    (wq_sb[:, ec, fc * 128:(fc + 1) * 128]),
                    (ag_sb[:, ec, :]),
                    start=(ec == 0), stop=(ec == EC - 1),
                )
            # block-diag pack (+ q bias): rows 0-63 head 2fc -> cols 0-31,
            # rows 64-127 head 2fc+1 -> cols 32-63
            nc.scalar.activation(qb_sb[0:64, fc, 0:RATIO], qp[0:64, :], AF.Identity,
                                 bias=bq_sb[0:64, fc, :])
            nc.scalar.activation(qb_sb[64:128, fc, RATIO:64], qp[64:128, :], AF.Identity,
                                 bias=bq_sb[64:128, fc, :])
            nc.scalar.activation(qb_sb[0:64, fc, RATIO:64], qp[0:64, :], AF.Identity,
                                 scale=0.0)
            nc.scalar.activation(qb_sb[64:128, fc, 0:RATIO], qp[64:128, :], AF.Identity,
                                 scale=0.0)

        # ---- K projection: KT[f,s] (f-chunk fc, s-half sh) ---------------
        for fc in range(FC):
            for sh in range(2):
                kp = ps.tile([128, 512], F32, tag="mm512", name=f"kp{fc}{sh}")
                for ec in range(EC):
                    nc.tensor.matmul(
                        kp[:],
                        (wk_sb[:, ec, fc * 128:(fc + 1) * 128]),
                        (x_sb[:, ec, sh * 512:(sh + 1) * 512]),
                        start=(ec == 0), stop=(ec == EC - 1),
                    )
                nc.scalar.copy(kt_sb[:, fc, sh * 512:(sh + 1) * 512], kp[:])

        # ---- V projection: V[s,f] per graph ------------------------------
        for g in range(G):
            vp = ps.tile([128, 512], F32, tag="mm512", name=f"vp{g}")
            for ec in range(EC):
                nc.tensor.matmul(
                    vp[:],
                    (x_sb[:, ec, g * 128:(g + 1) * 128]),
                    (wv_sb[:, ec, :]),
                    start=(ec == 0), stop=(ec == EC - 1),
                )
            nc.vector.tensor_copy(v_sb[:, g, :], vp[:])

        # ---- scores + softmax + transpose + attention, sh-major ----------
        # scores tile t=(hgrp, sh): rows = 4 heads x 32 q (2 matmul halves),
        # cols = 4 graphs x 128 nodes. After both hgrps of an sh-group are
        # transposed, attention + output projection for that graph-group run
        # while the next sh-group's scores occupy the PE.
        for sh in range(2):
            for hgrp in range(2):
                t = hgrp * 2 + sh
                for half in range(2):
                    fc = hgrp * 2 + half
                    sp = ps.tile([64, 512], F32, tag="mm512", name=f"sp{t}{half}")
                    nc.tensor.matmul(
                        sp[:],
                        (qb_sb[:, fc, :]),
                        (kt_sb[:, fc, sh * 512:(sh + 1) * 512]),
                        start=True, stop=True,
                    )
                    nc.scalar.activation(
                        ex_sb[half * 64:(half + 1) * 64, t, :], sp[:], AF.Exp)
                nc.vector.reduce_sum(
                    den_sb[:, t, :],
                    ex_sb[:, t, :].rearrange("p (j n) -> p j n", n=128),
                    axis=mybir.AxisListType.X,
                )
                nc.vector.reciprocal(rec_sb[:, t, :], den_sb[:, t, :])
                for j in range(4):
                    g = sh * 4 + j
                    nc.vector.tensor_scalar_mul(
                        ex_sb[:, t, j * 128:(j + 1) * 128],
                        ex_sb[:, t, j * 128:(j + 1) * 128],
                        rec_sb[:, t, j:j + 1],
                    )
                    tp = ps2.tile([128, 128], F32, tag="mm128", name=f"tp{t}{j}")
                    nc.tensor.transpose(tp[:].bitcast(F32R),
                                        (ex_sb[:, t, j * 128:(j + 1) * 128]),
                                        (id_sb[:]))
                    if (hgrp + j) % 2 == 0:
                        nc.vector.tensor_copy(at_sb[:, hgrp, g, :], tp[:])
                    else:
                        nc.scalar.copy(at_sb[:, hgrp, g, :], tp[:])

            # attention output for this graph-group:
            # yT[f=(2 heads x 64 d), (4 g x 32 q)] per head-pair
            gg = sh
            for hp in range(FC):          # head-pair hp: heads (2hp, 2hp+1)
                for hh in range(2):
                    h = 2 * hp + hh
                    hgrp, hl = h // 4, h % 4
                    yp = ps2.tile([64, 128], F32, tag="mm128", name=f"yp{gg}{h}")
                    for jg in range(4):
                        g = gg * 4 + jg
                        nc.tensor.matmul(
                            yp[:, jg * 32:(jg + 1) * 32],
                            (v_sb[:, g, h * 64:(h + 1) * 64]),
                            (at_sb[:, hgrp, g, hl * 32:(hl + 1) * 32]),
                            start=True, stop=True,
                        )
                    if hh == 0:
                        nc.vector.tensor_copy(
                            y_sb[hh * 64:(hh + 1) * 64, hp, gg, :], yp[:])
                    else:
                        nc.scalar.copy(
                            y_sb[hh * 64:(hh + 1) * 64, hp, gg, :], yp[:])

        # ---- output projection + bias ------------------------------------
        for gg in range(2):
            op = ps.tile([128, 512], F32, tag="mm512", name=f"op{gg}")
            for hp in range(FC):
                nc.tensor.matmul(
                    op[:], (y_sb[:, hp, gg, :]), (wo_sb[:, hp, :]),
                    start=(hp == 0), stop=False,
                )
            nc.tensor.matmul(op[:], (ones_sb[:]), (bo_sb[:]),
                             start=False, stop=True)
            nc.vector.tensor_copy(o_sb[:, gg, :], op[:])
            nc.sync.dma_start(out=d["out"][gg * 128:(gg + 1) * 128, :],
                              in_=o_sb[:, gg, :])


def _build():
    nc = bacc.Bacc("TRN2", target_bir_lowering=False, debug=False)
    d = {}
    d["xT"] = nc.dram_tensor("xT", (E, S), F32R, kind="ExternalInput").ap()
    d["wkT"] = nc.dram_tensor("wkT", (E, E), F32R, kind="ExternalInput").ap()
    d["wvT"] = nc.dram_tensor("wvT", (E, E), F32R, kind="ExternalInput").ap()
    d["wqT"] = nc.dram_tensor("wqT", (E, E), F32R, kind="ExternalInput").ap()
    d["woT"] = nc.dram_tensor("woT", (E, E), F32R, kind="ExternalInput").ap()
    d["agT"] = nc.dram_tensor("agT", (E, RATIO), F32R, kind="ExternalInput").ap()
    d["bq"] = nc.dram_tensor("bq", (E, 1), F32, kind="ExternalInput").ap()
    d["bo"] = nc.dram_tensor("bo", (1, E), F32R, kind="ExternalInput").ap()
    d["ident"] = nc.dram_tensor("ident", (128, 128), F32R, kind="ExternalInput").ap()
    d["ones"] = nc.dram_tensor("ones", (1, 128), F32R, kind="ExternalInput").ap()
    d["out"] = nc.dram_tensor("out", (G * RATIO, E), F32, kind="ExternalOutput").ap()
    with tile.TileContext(nc) as tc:
        _emit(nc, tc, d)
    nc.compile()
    return nc


def kernel(x, batch, aggrs, in_proj_w, in_proj_b, out_proj_w, out_proj_b):
    global LAST_RESULT
    x = np.ascontiguousarray(np.asarray(x, dtype=np.float32))
    aggrs = np.asarray(aggrs, dtype=np.float32)
    in_proj_w = np.asarray(in_proj_w, dtype=np.float32)
    in_proj_b = np.asarray(in_proj_b, dtype=np.float32)
    out_proj_w = np.asarray(out_proj_w, dtype=np.float32)
    out_proj_b = np.asarray(out_proj_b, dtype=np.float32)

    scale = np.float32(1.0 / np.sqrt(HD))
    wq, wk, wv = in_proj_w[:E], in_proj_w[E:2 * E], in_proj_w[2 * E:]
    bq = in_proj_b[:E] * scale
    bv = in_proj_b[2 * E:]
    wqT = np.ascontiguousarray((wq * scale).T)
    wkT = np.ascontiguousarray(wk.T)
    wvT = np.ascontiguousarray(wv.T)
    woT = np.ascontiguousarray(out_proj_w.T)
    agT = np.ascontiguousarray(aggrs.T)
    bo_eff = (out_proj_w @ bv + out_proj_b).reshape(1, E)
    ident = np.eye(128, dtype=np.float32)

    shared = {
        "wkT": wkT, "wvT": wvT, "wqT": wqT, "woT": woT, "agT": agT,
        "bq": bq.reshape(E, 1).astype(np.float32),
        "bo": bo_eff.astype(np.float32),
        "ident": ident,
        "ones": np.ones((1, 128), dtype=np.float32),
    }
    in_maps = []
    for c in range(NCORES):
        xc = x[c * G:(c + 1) * G].reshape(S, E)
        m = dict(shared)
        m["xT"] = np.ascontiguousarray(xc.T)
        in_maps.append(m)

    if "nc" not in _CACHE:
        _CACHE["nc"] = _build()
    nc = _CACHE["nc"]

    res = run_bass_kernel_spmd(nc, in_maps, list(range(NCORES)))
    LAST_RESULT = res
    out = np.concatenate([res.results[c]["out"] for c in range(NCORES)], axis=0)
    return out.reshape(B, RATIO, E).astype(np.float32)


# revision 11
# speedup vs baseline: 1.1466x; 1.0245x over previous
"""Bass/Trainium2 kernel for nn_BipartPool: bipartite attention pooling.

Math (B=64 graphs, N=128 nodes/graph, R=32 aggregator queries/graph,
H=8 heads, HD=64, E=512):
  q = (aggrs @ Wq.T + bq)            -- identical for every graph
  k = x @ Wk.T, v = x @ Wv.T          (per node)
  per graph g, head h: attn = softmax(q_h k_{g,h}^T / sqrt(HD))
  out_g = concat_h(attn @ v_{g,h}) @ Wo.T + bo

Sharding: data-parallel over graphs, 8 graphs per core across 8 cores.
Weights replicated. No collectives.

Simplifications used (all mathematically exact):
  - k-bias bk drops out (softmax shift invariance along the node axis).
  - v-bias bv folds into the output bias: bo_eff = Wo @ bv + bo.
  - 1/sqrt(HD) folds into Wq and bq host-side.
  - softmax without max-subtraction (scores are ~N(0,1); exp is safe in fp32).

Device layout per core (G=8 graphs, S=G*128=1024 nodes):
  xT   [E, S]  (host-transposed)   KT = Wk @ xT   [E_f, S]
  V    [S, E]  (natural)           QT  [E_f, 32]
  Qblk [128, 64] per f-chunk: block-diag pair of per-head qT -> one
      matmul per (head-pair, s-half) produces scores for 2 heads.
  softmax rows = (head, query) on partitions, nodes on free axis.
  PE-transpose of normalized attn -> attnT [node, (4 heads x 32 q)].
  outT_h = v_{g,h}.T-free matmul(lhsT=v, rhs=attnT slice) -> yT chunks.
  out = yT.T @ WoT (+ ones x bo_eff outer product) -> [256, 512] per core.
"""

import numpy as np

import concourse.bacc as bacc
import concourse.mybir as mybir
from concourse import tile
from concourse.bass_utils import run_bass_kernel_spmd

F32 = mybir.dt.float32
F32R = mybir.dt.float32r
AF = mybir.ActivationFunctionType

B, N, RATIO, H, HD = 64, 128, 32, 8, 64
E = H * HD                 # 512
NCORES = 8
G = B // NCORES            # 8 graphs per core
S = G * N                  # 1024 nodes per core
EC = E // 128              # 4 e-chunks (contraction)
FC = E // 128              # 4 f-chunks (output features)

_CACHE = {}
LAST_RESULT = None         # test harness reads exec_time_ns from here


def _r(ap):
    return ap.bitcast(F32R)


def _emit(nc, tc, d):
    """Emit the per-core program. d: dict of dram APs."""
    with (
        nc.allow_low_precision(reason="float32r rounding is intended"),
        tc.tile_pool(name="sb", bufs=1) as sb,
        tc.tile_pool(name="ps", bufs=4, space="PSUM") as ps,
        tc.tile_pool(name="ps2", bufs=4, space="PSUM") as ps2,
    ):
        # ---- persistent SBUF tensors -------------------------------------
        x_sb = sb.tile([128, EC, S], F32R)          # xT  [e-part, ec, s]
        wk_sb = sb.tile([128, EC, E], F32R)         # WkT [e-part, ec, f]
        wv_sb = sb.tile([128, EC, E], F32R)
        wq_sb = sb.tile([128, EC, E], F32R)
        wo_sb = sb.tile([128, FC, E], F32R)         # WoT [f-part, fc, e]
        ag_sb = sb.tile([128, EC, RATIO], F32R)     # aggrsT [e-part, ec, q]
        bq_sb = sb.tile([128, FC, 1], F32)
        bo_sb = sb.tile([1, E], F32R)
        id_sb = sb.tile([128, 128], F32R)           # identity for PE transpose
        kt_sb = sb.tile([128, FC, S], F32R)         # KT [f-part, fc, s]
        v_sb = sb.tile([128, G, E], F32R)           # V  [node, g, f]
        qb_sb = sb.tile([128, FC, 64], F32R)        # block-diag head-pair qT
        ex_sb = sb.tile([128, 4, 512], F32R)        # exp(scores) (hgrp, sh)
        at_sb = sb.tile([128, 2, G, 128], F32R)     # attnT (hgrp, g)
        y_sb = sb.tile([128, FC, 2, 128], F32R)     # yT (head-pair, gg)
        o_sb = sb.tile([128, 2, E], F32)           # output rows
        den_sb = sb.tile([128, 4, 4], F32)         # softmax denominators
        rec_sb = sb.tile([128, 4, 4], F32)
        ones_sb = sb.tile([1, 128], F32R)

        # ---- DMA in ------------------------------------------------------
        # Chunked, priority-ordered loads on the three DMA-capable queues so
        # the first projection matmuls start after ~1.5 MB instead of 6.3 MB.
        nc.scalar.dma_start(out=ag_sb[:], in_=d["agT"].rearrange("(c p) q -> p c q", p=128))
        nc.scalar.dma_start(out=bq_sb[:], in_=d["bq"].rearrange("(c p) o -> p c o", p=128))
        for ec in range(EC):
            nc.sync.dma_start(out=x_sb[:, ec, :], in_=d["xT"][ec * 128:(ec + 1) * 128, :])
            nc.scalar.dma_start(out=wk_sb[:, ec, :], in_=d["wkT"][ec * 128:(ec + 1) * 128, :])
            nc.gpsimd.dma_start(out=wv_sb[:, ec, :], in_=d["wvT"][ec * 128:(ec + 1) * 128, :])
        for ec in range(EC):
            nc.scalar.dma_start(out=wq_sb[:, ec, :], in_=d["wqT"][ec * 128:(ec + 1) * 128, :])
        nc.sync.dma_start(out=id_sb[:], in_=d["ident"][:])
        for fc in range(FC):
            nc.gpsimd.dma_start(out=wo_sb[:, fc, :], in_=d["woT"][fc * 128:(fc + 1) * 128, :])
        nc.sync.dma_start(out=bo_sb[:], in_=d["bo"][:])
        nc.sync.dma_start(out=ones_sb[:], in_=d["ones"][:])

        # ---- K projection: KT[f,s] (f-chunk fc, s-half sh) ---------------
        for fc in range(FC):
            for sh in range(2):
                kp = ps.tile([128, 512], F32, tag="mm512", name=f"kp{fc}{sh}")
                for ec in range(EC):
                    nc.tensor.matmul(
                        kp[:],
                        (wk_sb[:, ec, fc * 128:(fc + 1) * 128]),
                        (x_sb[:, ec, sh * 512:(sh + 1) * 512]),
                        start=(ec == 0), stop=(ec == EC - 1),
                    )
                nc.scalar.copy(kt_sb[:, fc, sh * 512:(sh + 1) * 512], kp[:])

        # ---- Q projection: QT[f,q] = sum_e WqT[e,f] agT[e,q] -------------
        for fc in range(FC):
            qp = ps.tile([128, RATIO], F32, tag="mm512", name=f"qp{fc}")
            for ec in range(EC):
                nc.tensor.matmul(
                    qp[:],
                    (wq_sb[:, ec, fc * 128:(fc + 1) * 128]),
                    (ag_sb[:, ec, :]),
                    start=(ec == 0), stop=(ec == EC - 1),
                )
            # block-diag pack (+ q bias): rows 0-63 head 2fc -> cols 0-31,
            # rows 64-127 head 2fc+1 -> cols 32-63
            nc.scalar.activation(qb_sb[0:64, fc, 0:RATIO], qp[0:64, :], AF.Identity,
                                 bias=bq_sb[0:64, fc, :])
            nc.scalar.activation(qb_sb[64:128, fc, RATIO:64], qp[64:128, :], AF.Identity,
                                 bias=bq_sb[64:128, fc, :])
            nc.scalar.activation(qb_sb[0:64, fc, RATIO:64], qp[0:64, :], AF.Identity,
                                 scale=0.0)
            nc.scalar.activation(qb_sb[64:128, fc, 0:RATIO], qp[64:128, :], AF.Identity,
                                 scale=0.0)

        # ---- V projection: V[s,f] per graph ------------------------------
        for g in range(G):
            vp = ps.tile([128, 512], F32, tag="mm512", name=f"vp{g}")
            for ec in range(EC):
                nc.tensor.matmul(
                    vp[:],
                    (x_sb[:, ec, g * 128:(g + 1) * 128]),
                    (wv_sb[:, ec, :]),
                    start=(ec == 0), stop=(ec == EC - 1),
                )
            nc.vector.tensor_copy(v_sb[:, g, :], vp[:])

        # ---- scores + softmax + transpose + attention, sh-major ----------
        # scores tile t=(hgrp, sh): rows = 4 heads x 32 q (2 matmul halves),
        # cols = 4 graphs x 128 nodes. After both hgrps of an sh-group are
        # transposed, attention + output projection for that graph-group run
        # while the next sh-group's scores occupy the PE.
        for sh in range(2):
            for hgrp in range(2):
                t = hgrp * 2 + sh
                for half in range(2):
                    fc = hgrp * 2 + half
                    sp = ps.tile([64, 512], F32, tag="mm512", name=f"sp{t}{half}")
                    nc.tensor.matmul(
                        sp[:],
                        (qb_sb[:, fc, :]),
                        (kt_sb[:, fc, sh * 512:(sh + 1) * 512]),
                        start=True, stop=True,
                    )
                    nc.scalar.activation(
                        ex_sb[half * 64:(half + 1) * 64, t, :], sp[:], AF.Exp)
                nc.vector.reduce_sum(
                    den_sb[:, t, :],
                    ex_sb[:, t, :].rearrange("p (j n) -> p j n", n=128),
                    axis=mybir.AxisListType.X,
                )
                nc.vector.reciprocal(rec_sb[:, t, :], den_sb[:, t, :])
                for j in range(4):
                    g = sh * 4 + j
                    nc.vector.tensor_scalar_mul(
                        ex_sb[:, t, j * 128:(j + 1) * 128],
                        ex_sb[:, t, j * 128:(j + 1) * 128],
                        rec_sb[:, t, j:j + 1],
                    )
                    tp = ps2.tile([128, 128], F32, tag="mm128", name=f"tp{t}{j}")
                    nc.tensor.transpose(tp[:].bitcast(F32R),
                                        (ex_sb[:, t, j * 128:(j + 1) * 128]),
                                        (id_sb[:]))
                    if (hgrp + j) % 2 == 0:
                        nc.vector.tensor_copy(at_sb[:, hgrp, g, :], tp[:])
                    else:
                        nc.scalar.copy(at_sb[:, hgrp, g, :], tp[:])

            # attention output for this graph-group:
            # yT[f=(2 heads x 64 d), (4 g x 32 q)] per head-pair
            gg = sh
            for hp in range(FC):          # head-pair hp: heads (2hp, 2hp+1)
                for hh in range(2):
                    h = 2 * hp + hh
                    hgrp, hl = h // 4, h % 4
                    yp = ps2.tile([64, 128], F32, tag="mm128", name=f"yp{gg}{h}")
                    for jg in range(4):
                        g = gg * 4 + jg
                        nc.tensor.matmul(
                            yp[:, jg * 32:(jg + 1) * 32],
                            (v_sb[:, g, h * 64:(h + 1) * 64]),
                            (at_sb[:, hgrp, g, hl * 32:(hl + 1) * 32]),
                            start=True, stop=True,
                        )
                    if hh == 0:
                        nc.vector.tensor_copy(
                            y_sb[hh * 64:(hh + 1) * 64, hp, gg, :], yp[:])
                    else:
                        nc.scalar.copy(
                            y_sb[hh * 64:(hh + 1) * 64, hp, gg, :], yp[:])

        # ---- output projection + bias ------------------------------------
        for gg in range(2):
            op = ps.tile([128, 512], F32, tag="mm512", name=f"op{gg}")
            for hp in range(FC):
                nc.tensor.matmul(
                    op[:], (y_sb[:, hp, gg, :]), (wo_sb[:, hp, :]),
                    start=(hp == 0), stop=False,
                )
            nc.tensor.matmul(op[:], (ones_sb[:]), (bo_sb[:]),
                             start=False, stop=True)
            nc.vector.tensor_copy(o_sb[:, gg, :], op[:])
            nc.sync.dma_start(out=d["out"][gg * 128:(gg + 1) * 128, :],
                              in_=o_sb[:, gg, :])


def _build():
    nc = bacc.Bacc("TRN2", target_bir_lowering=False, debug=False)
    d = {}
    d["xT"] = nc.dram_tensor("xT", (E, S), F32R, kind="ExternalInput").ap()
    d["wkT"] = nc.dram_tensor("wkT", (E, E), F32R, kind="ExternalInput").ap()
    d["wvT"] = nc.dram_tensor("wvT", (E, E), F32R, kind="ExternalInput").ap()
    d["wqT"] = nc.dram_tensor("wqT", (E, E), F32R, kind="ExternalInput").ap()
    d["woT"] = nc.dram_tensor("woT", (E, E), F32R, kind="ExternalInput").ap()
    d["agT"] = nc.dram_tensor("agT", (E, RATIO), F32R, kind="ExternalInput").ap()
    d["bq"] = nc.dram_tensor("bq", (E, 1), F32, kind="ExternalInput").ap()
    d["bo"] = nc.dram_tensor("bo", (1, E), F32R, kind="ExternalInput").ap()
    d["ident"] = nc.dram_tensor("ident", (128, 128), F32R, kind="ExternalInput").ap()
    d["ones"] = nc.dram_tensor("ones", (1, 128), F32R, kind="ExternalInput").ap()
    d["out"] = nc.dram_tensor("out", (G * RATIO, E), F32, kind="ExternalOutput").ap()
    with tile.TileContext(nc) as tc:
        _emit(nc, tc, d)
    nc.compile()
    return nc


def kernel(x, batch, aggrs, in_proj_w, in_proj_b, out_proj_w, out_proj_b):
    global LAST_RESULT
    x = np.ascontiguousarray(np.asarray(x, dtype=np.float32))
    aggrs = np.asarray(aggrs, dtype=np.float32)
    in_proj_w = np.asarray(in_proj_w, dtype=np.float32)
    in_proj_b = np.asarray(in_proj_b, dtype=np.float32)
    out_proj_w = np.asarray(out_proj_w, dtype=np.float32)
    out_proj_b = np.asarray(out_proj_b, dtype=np.float32)

    scale = np.float32(1.0 / np.sqrt(HD))
    wq, wk, wv = in_proj_w[:E], in_proj_w[E:2 * E], in_proj_w[2 * E:]
    bq = in_proj_b[:E] * scale
    bv = in_proj_b[2 * E:]
    wqT = np.ascontiguousarray((wq * scale).T)
    wkT = np.ascontiguousarray(wk.T)
    wvT = np.ascontiguousarray(wv.T)
    woT = np.ascontiguousarray(out_proj_w.T)
    agT = np.ascontiguousarray(aggrs.T)
    bo_eff = (out_proj_w @ bv + out_proj_b).reshape(1, E)
    ident = np.eye(128, dtype=np.float32)

    shared = {
        "wkT": wkT, "wvT": wvT, "wqT": wqT, "woT": woT, "agT": agT,
        "bq": bq.reshape(E, 1).astype(np.float32),
        "bo": bo_eff.astype(np.float32),
        "ident": ident,
        "ones": np.ones((1, 128), dtype=np.float32),
    }
    in_maps = []
    for c in range(NCORES):
        xc = x[c * G:(c + 1) * G].reshape(S, E)
        m = dict(shared)
        m["xT"] = np.ascontiguousarray(xc.T)
        in_maps.append(m)

    if "nc" not in _CACHE:
        _CACHE["nc"] = _build()
    nc = _CACHE["nc"]

    res = run_bass_kernel_spmd(nc, in_maps, list(range(NCORES)))
    LAST_RESULT = res
    out = np.concatenate([res.results[c]["out"] for c in range(NCORES)], axis=0)
    return out.reshape(B, RATIO, E).astype(np.float32)


# revision 12
# speedup vs baseline: 1.1961x; 1.0432x over previous
"""Bass/Trainium2 kernel for nn_BipartPool: bipartite attention pooling.

Math (B=64 graphs, N=128 nodes/graph, R=32 aggregator queries/graph,
H=8 heads, HD=64, E=512):
  q = (aggrs @ Wq.T + bq)            -- identical for every graph
  k = x @ Wk.T, v = x @ Wv.T          (per node)
  per graph g, head h: attn = softmax(q_h k_{g,h}^T / sqrt(HD))
  out_g = concat_h(attn @ v_{g,h}) @ Wo.T + bo

Sharding: data-parallel over graphs, 8 graphs per core across 8 cores.
Weights replicated. No collectives.

Simplifications used (all mathematically exact):
  - k-bias bk drops out (softmax shift invariance along the node axis).
  - v-bias bv folds into the output bias: bo_eff = Wo @ bv + bo.
  - 1/sqrt(HD) folds into Wq and bq host-side.
  - softmax without max-subtraction (scores are ~N(0,1); exp is safe in fp32).

Device layout per core (G=8 graphs, S=G*128=1024 nodes):
  xT   [E, S]  (host-transposed)   KT = Wk @ xT   [E_f, S]
  V    [S, E]  (natural)           QT  [E_f, 32]
  Qblk [128, 64] per f-chunk: block-diag pair of per-head qT -> one
      matmul per (head-pair, s-half) produces scores for 2 heads.
  softmax rows = (head, query) on partitions, nodes on free axis.
  PE-transpose of normalized attn -> attnT [node, (4 heads x 32 q)].
  outT_h = v_{g,h}.T-free matmul(lhsT=v, rhs=attnT slice) -> yT chunks.
  out = yT.T @ WoT (+ ones x bo_eff outer product) -> [256, 512] per core.
"""

import numpy as np

import concourse.bacc as bacc
import concourse.mybir as mybir
from concourse import tile
from concourse.bass_utils import run_bass_kernel_spmd

F32 = mybir.dt.float32
F32R = mybir.dt.float32r
AF = mybir.ActivationFunctionType

B, N, RATIO, H, HD = 64, 128, 32, 8, 64
E = H * HD                 # 512
NCORES = 8
G = B // NCORES            # 8 graphs per core
S = G * N                  # 1024 nodes per core
EC = E // 128              # 4 e-chunks (contraction)
FC = E // 128              # 4 f-chunks (output features)

_CACHE = {}
LAST_RESULT = None         # test harness reads exec_time_ns from here


def _r(ap):
    return ap.bitcast(F32R)


def _emit(nc, tc, d):
    """Emit the per-core program. d: dict of dram APs."""
    with (
        nc.allow_low_precision(reason="float32r rounding is intended"),
        tc.tile_pool(name="sb", bufs=1) as sb,
        tc.tile_pool(name="ps", bufs=4, space="PSUM") as ps,
        tc.tile_pool(name="ps2", bufs=4, space="PSUM") as ps2,
    ):
        # ---- persistent SBUF tensors -------------------------------------
        x_sb = sb.tile([128, EC, S], F32R)          # xT  [e-part, ec, s]
        wk_sb = sb.tile([128, FC, E], F32R)         # Wk natural [f-part, fc, e]
        wv_sb = sb.tile([128, EC, E], F32R)
        wq_sb = sb.tile([128, EC, E], F32R)
        wo_sb = sb.tile([128, FC, E], F32R)         # WoT [f-part, fc, e]
        ag_sb = sb.tile([128, EC, RATIO], F32R)     # aggrsT [e-part, ec, q]
        bq_sb = sb.tile([128, FC, 1], F32)
        bo_sb = sb.tile([1, E], F32R)
        id_sb = sb.tile([128, 128], F32R)           # identity for PE transpose
        v_sb = sb.tile([128, G, E], F32R)           # V  [node, g, f]
        qb_sb = sb.tile([128, FC, 256], F32R)       # block-diag qT, 8 head blocks
        a_sb = sb.tile([128, EC, 256], F32R)        # A^T = Wk.T Qblk [e, (h q)]
        ex_sb = sb.tile([128, 4, 512], F32R)        # exp(scores) (hgrp, sh)
        at_sb = sb.tile([128, 2, G, 128], F32R)     # attnT (hgrp, g)
        y_sb = sb.tile([128, FC, 2, 128], F32R)     # yT (head-pair, gg)
        o_sb = sb.tile([128, 2, E], F32)           # output rows
        den_sb = sb.tile([128, 4, 4], F32)         # softmax denominators
        rec_sb = sb.tile([128, 4, 4], F32)
        ones_sb = sb.tile([1, 128], F32R)

        # ---- DMA in ------------------------------------------------------
        # Chunked, priority-ordered loads on the three DMA-capable queues.
        # The Q -> Qblk -> A chain starts as soon as wq lands, so the PE has
        # work within a few us while x / wv stream in.
        nc.scalar.dma_start(out=ag_sb[:], in_=d["agT"].rearrange("(c p) q -> p c q", p=128))
        nc.scalar.dma_start(out=bq_sb[:], in_=d["bq"].rearrange("(c p) o -> p c o", p=128))
        for ec in range(EC):
            nc.scalar.dma_start(out=wq_sb[:, ec, :], in_=d["wqT"][ec * 128:(ec + 1) * 128, :])
            nc.gpsimd.dma_start(out=wk_sb[:, ec, :], in_=d["wk"][ec * 128:(ec + 1) * 128, :])
            nc.sync.dma_start(out=x_sb[:, ec, :], in_=d["xT"][ec * 128:(ec + 1) * 128, :])
        for ec in range(EC):
            nc.scalar.dma_start(out=wv_sb[:, ec, :], in_=d["wvT"][ec * 128:(ec + 1) * 128, :])
        nc.sync.dma_start(out=id_sb[:], in_=d["ident"][:])
        for fc in range(FC):
            nc.gpsimd.dma_start(out=wo_sb[:, fc, :], in_=d["woT"][fc * 128:(fc + 1) * 128, :])
        nc.sync.dma_start(out=bo_sb[:], in_=d["bo"][:])
        nc.sync.dma_start(out=ones_sb[:], in_=d["ones"][:])

        # ---- Q projection + block-diag pack + A^T = Wk.T @ Qblk ----------
        # Qblk [f, (h q)]: head h occupies rows 64h..64h+63 (within chunk
        # fc=h//2) and cols 32h..32h+31; zero elsewhere. Then
        # a_hq = Wk_h.T q'_hq collapses the K projection into the query side:
        # scores = A^T.T @ xT directly.
        for fc in range(FC):
            qp = ps.tile([128, RATIO], F32, tag="mm512", name=f"qp{fc}")
            for ec in range(EC):
                nc.tensor.matmul(
                    qp[:],
                    (wq_sb[:, ec, fc * 128:(fc + 1) * 128]),
                    (ag_sb[:, ec, :]),
                    start=(ec == 0), stop=(ec == EC - 1),
                )
            h0, h1 = 2 * fc, 2 * fc + 1
            # zero the full chunk rows first (scale=0), then write the blocks
            nc.scalar.activation(qb_sb[0:64, fc, :], wq_sb[0:64, fc, 0:256],
                                 AF.Identity, scale=0.0)
            nc.scalar.activation(qb_sb[64:128, fc, :], wq_sb[64:128, fc, 0:256],
                                 AF.Identity, scale=0.0)
            nc.scalar.activation(qb_sb[0:64, fc, h0 * 32:h0 * 32 + 32],
                                 qp[0:64, :], AF.Identity, bias=bq_sb[0:64, fc, :])
            nc.scalar.activation(qb_sb[64:128, fc, h1 * 32:h1 * 32 + 32],
                                 qp[64:128, :], AF.Identity, bias=bq_sb[64:128, fc, :])
        for eoc in range(EC):
            ap_ = ps.tile([128, 256], F32, tag="mm512", name=f"ap{eoc}")
            for fcc in range(FC):
                nc.tensor.matmul(
                    ap_[:],
                    (wk_sb[:, fcc, eoc * 128:(eoc + 1) * 128]),
                    (qb_sb[:, fcc, :]),
                    start=(fcc == 0), stop=(fcc == FC - 1),
                )
            nc.vector.tensor_copy(a_sb[:, eoc, :], ap_[:])

        # ---- V projection: V[s,f] per graph ------------------------------
        for g in range(G):
            vp = ps.tile([128, 512], F32, tag="mm512", name=f"vp{g}")
            for ec in range(EC):
                nc.tensor.matmul(
                    vp[:],
                    (x_sb[:, ec, g * 128:(g + 1) * 128]),
                    (wv_sb[:, ec, :]),
                    start=(ec == 0), stop=(ec == EC - 1),
                )
            nc.vector.tensor_copy(v_sb[:, g, :], vp[:])

        # ---- scores + softmax + transpose + attention, sh-major ----------
        # scores tile t=(hgrp, sh): rows = 4 heads x 32 q (2 matmul halves),
        # cols = 4 graphs x 128 nodes. After both hgrps of an sh-group are
        # transposed, attention + output projection for that graph-group run
        # while the next sh-group's scores occupy the PE.
        for sh in range(2):
            for hgrp in range(2):
                t = hgrp * 2 + sh
                sp = ps.tile([128, 512], F32, tag="mm512", name=f"sp{t}")
                for ec in range(EC):
                    nc.tensor.matmul(
                        sp[:],
                        (a_sb[:, ec, hgrp * 128:(hgrp + 1) * 128]),
                        (x_sb[:, ec, sh * 512:(sh + 1) * 512]),
                        start=(ec == 0), stop=(ec == EC - 1),
                    )
                nc.scalar.activation(ex_sb[:, t, :], sp[:], AF.Exp)
                nc.vector.reduce_sum(
                    den_sb[:, t, :],
                    ex_sb[:, t, :].rearrange("p (j n) -> p j n", n=128),
                    axis=mybir.AxisListType.X,
                )
                nc.vector.reciprocal(rec_sb[:, t, :], den_sb[:, t, :])
                for j in range(4):
                    g = sh * 4 + j
                    nc.vector.tensor_scalar_mul(
                        ex_sb[:, t, j * 128:(j + 1) * 128],
                        ex_sb[:, t, j * 128:(j + 1) * 128],
                        rec_sb[:, t, j:j + 1],
                    )
                    tp = ps2.tile([128, 128], F32, tag="mm128", name=f"tp{t}{j}")
                    nc.tensor.transpose(tp[:].bitcast(F32R),
                                        (ex_sb[:, t, j * 128:(j + 1) * 128]),
                                        (id_sb[:]))
                    if (hgrp + j) % 2 == 0:
                        nc.vector.tensor_copy(at_sb[:, hgrp, g, :], tp[:])
                    else:
                        nc.scalar.copy(at_sb[:, hgrp, g, :], tp[:])

            # attention output for this graph-group:
            # yT[f=(2 heads x 64 d), (4 g x 32 q)] per head-pair
            gg = sh
            for hp in range(FC):          # head-pair hp: heads (2hp, 2hp+1)
                for hh in range(2):
                    h = 2 * hp + hh
                    hgrp, hl = h // 4, h % 4
                    yp = ps2.tile([64, 128], F32, tag="mm128", name=f"yp{gg}{h}")
                    for jg in range(4):
                        g = gg * 4 + jg
                        nc.tensor.matmul(
                            yp[:, jg * 32:(jg + 1) * 32],
                            (v_sb[:, g, h * 64:(h + 1) * 64]),
                            (at_sb[:, hgrp, g, hl * 32:(hl + 1) * 32]),
                            start=True, stop=True,
                        )
                    if hh == 0:
                        nc.vector.tensor_copy(
                            y_sb[hh * 64:(hh + 1) * 64, hp, gg, :], yp[:])
                    else:
                        nc.scalar.copy(
                            y_sb[hh * 64:(hh + 1) * 64, hp, gg, :], yp[:])

        # ---- output projection + bias ------------------------------------
        for gg in range(2):
            op = ps.tile([128, 512], F32, tag="mm512", name=f"op{gg}")
            for hp in range(FC):
                nc.tensor.matmul(
                    op[:], (y_sb[:, hp, gg, :]), (wo_sb[:, hp, :]),
                    start=(hp == 0), stop=False,
                )
            nc.tensor.matmul(op[:], (ones_sb[:]), (bo_sb[:]),
                             start=False, stop=True)
            nc.vector.tensor_copy(o_sb[:, gg, :], op[:])
            nc.sync.dma_start(out=d["out"][gg * 128:(gg + 1) * 128, :],
                              in_=o_sb[:, gg, :])


def _build():
    nc = bacc.Bacc("TRN2", target_bir_lowering=False, debug=False)
    d = {}
    d["xT"] = nc.dram_tensor("xT", (E, S), F32R, kind="ExternalInput").ap()
    d["wk"] = nc.dram_tensor("wk", (E, E), F32R, kind="ExternalInput").ap()
    d["wvT"] = nc.dram_tensor("wvT", (E, E), F32R, kind="ExternalInput").ap()
    d["wqT"] = nc.dram_tensor("wqT", (E, E), F32R, kind="ExternalInput").ap()
    d["woT"] = nc.dram_tensor("woT", (E, E), F32R, kind="ExternalInput").ap()
    d["agT"] = nc.dram_tensor("agT", (E, RATIO), F32R, kind="ExternalInput").ap()
    d["bq"] = nc.dram_tensor("bq", (E, 1), F32, kind="ExternalInput").ap()
    d["bo"] = nc.dram_tensor("bo", (1, E), F32R, kind="ExternalInput").ap()
    d["ident"] = nc.dram_tensor("ident", (128, 128), F32R, kind="ExternalInput").ap()
    d["ones"] = nc.dram_tensor("ones", (1, 128), F32R, kind="ExternalInput").ap()
    d["out"] = nc.dram_tensor("out", (G * RATIO, E), F32, kind="ExternalOutput").ap()
    with tile.TileContext(nc) as tc:
        _emit(nc, tc, d)
    nc.compile()
    return nc


def kernel(x, batch, aggrs, in_proj_w, in_proj_b, out_proj_w, out_proj_b):
    global LAST_RESULT
    x = np.ascontiguousarray(np.asarray(x, dtype=np.float32))
    aggrs = np.asarray(aggrs, dtype=np.float32)
    in_proj_w = np.asarray(in_proj_w, dtype=np.float32)
    in_proj_b = np.asarray(in_proj_b, dtype=np.float32)
    out_proj_w = np.asarray(out_proj_w, dtype=np.float32)
    out_proj_b = np.asarray(out_proj_b, dtype=np.float32)

    scale = np.float32(1.0 / np.sqrt(HD))
    wq, wk, wv = in_proj_w[:E], in_proj_w[E:2 * E], in_proj_w[2 * E:]
    bq = in_proj_b[:E] * scale
    bv = in_proj_b[2 * E:]
    wqT = np.ascontiguousarray((wq * scale).T)
    wvT = np.ascontiguousarray(wv.T)
    woT = np.ascontiguousarray(out_proj_w.T)
    agT = np.ascontiguousarray(aggrs.T)
    bo_eff = (out_proj_w @ bv + out_proj_b).reshape(1, E)
    ident = np.eye(128, dtype=np.float32)

    shared = {
        "wk": np.ascontiguousarray(wk), "wvT": wvT, "wqT": wqT, "woT": woT, "agT": agT,
        "bq": bq.reshape(E, 1).astype(np.float32),
        "bo": bo_eff.astype(np.float32),
        "ident": ident,
        "ones": np.ones((1, 128), dtype=np.float32),
    }
    in_maps = []
    for c in range(NCORES):
        xc = x[c * G:(c + 1) * G].reshape(S, E)
        m = dict(shared)
        m["xT"] = np.ascontiguousarray(xc.T)
        in_maps.append(m)

    if "nc" not in _CACHE:
        _CACHE["nc"] = _build()
    nc = _CACHE["nc"]

    res = run_bass_kernel_spmd(nc, in_maps, list(range(NCORES)))
    LAST_RESULT = res
    out = np.concatenate([res.results[c]["out"] for c in range(NCORES)], axis=0)
    return out.reshape(B, RATIO, E).astype(np.float32)
